# revision 16
# baseline (speedup 1.0000x reference)
"""BitLinear (BitNet b1.58) forward kernel for Trainium2, 8 NeuronCores.

Computes  y = einsum('bsi,oi->bso', x, w_ste) + bias  where
  scale  = max(mean(|W|), 1e-8)
  w_q    = clip(round(W/scale), -1, 1)   (ternary)
  w_ste  = w_q * scale

Sharding: data-parallel over rows; each core owns one batch element
(2048 rows) and the full weight.

Quantization happens on the HOST, bit-exactly replicating the reference
(scale via jax-on-CPU mean — numpy's pairwise mean is 2 ulps off, which
flips ternary weights at the round(w/scale) boundary; with the exact
scale, numpy's round/clip reproduce the reference ternary identically).

Device: pure fp8 DoubleRow matmuls (2 contraction rows/cycle — the only
2x-rate PE mode; requires both operands fp8e4/e5; measured 216ns per
512-out-row MM = the 157 TF/s fp8 peak). All error then comes from
e4m3-quantizing x, reduced by a residual pass over the first R of 16
k-pairs: xr16 = e4m3(16*(x - x8)) against wr = ternary*(1/16),
accumulated into the same PSUM group. Host-simulated (exact predictor
of hw, verified): rel = 1.893e-2 at R=8 vs the 2e-2 gate.

Schedule: the PE floor is (16+R)/16 * 437us; everything else hides
behind it. Chunk 0 runs k-major across 8 PSUM banks (m-tiles 0-7) so
the PE starts as soon as k-pair 0 lands. To keep the k-major phase
under the per-core DMA roofline (~358 GB/s), x ships in two DRAM
tensors: x8a holds the mi 0-7 halves of each k-tile (tile-major, fed
in consumption order during the k-major phase at ~220 GB/s demand),
and x8b holds the mi 8-15 slices in MI-MAJOR layout so each m-tile of
the later m-major phase arrives as one 512KB DMA with 4KB-contiguous
per-partition packets. xr splits the same way (xra/xrb).

Head DMAs are byte-balanced across the two hardware DGE queues
(SP=nc.sync and Activation=nc.scalar) in consumption order, with the
x8b/xrb (mi 8-15) streams on the independent SWDGE ring (nc.gpsimd) so
the three descriptor streams together approach the per-core HBM rate;
steady state specializes (scalar: one big weight DMA per chunk; sync:
y writes). Drain: y16 = psum * scale + bias fused on DVE, written fp16
(upcast to f32 on host; fp16 rounding adds ~2e-4 rel).
"""

import numpy as np
import ml_dtypes

import concourse.tile as tile
import concourse.mybir as mybir
from concourse import bacc
from concourse.bass import ts
from concourse.bass_utils import run_bass_kernel_spmd

N_CORES = 8
IN_F = 4096
OUT_F = 4096
ROWS = 2048               # rows per core
P = 128                   # SBUF partitions
KT = IN_F // P            # 32 k-tiles
KP = KT // 2              # 16 k-pairs (DoubleRow covers 2 k-tiles)
MT = ROWS // P            # 16 row-tiles per core
OCH = 512                 # out-feature chunk = PSUM bank width
NCH = OUT_F // OCH        # 8 chunks
R = 8                     # residual k-pairs covered (of 16)
HR = ROWS // 2            # 1024: columns of the mi 0-7 half of a k-tile

F32 = mybir.dt.float32
F16 = mybir.dt.float16
F8 = mybir.dt.float8e4
E4 = np.dtype(ml_dtypes.float8_e4m3)
DR = mybir.MatmulPerfMode.DoubleRow

LAST_RESULTS = None
_NC_CACHE = {}


def _build():
    nc = bacc.Bacc(
        "TRN2", target_bir_lowering=False, debug=False, num_devices=N_CORES
    )
    # partition-major layouts (second dim is per-partition linear bytes)
    x8a = nc.dram_tensor("x8a", [P, KT * HR], F8, kind="ExternalInput").ap()
    x8b = nc.dram_tensor("x8b", [P, 8 * KT * P], F8, kind="ExternalInput").ap()
    xra = nc.dram_tensor("xra", [P, 2 * R * HR], F8, kind="ExternalInput").ap()
    xrb = nc.dram_tensor("xrb", [P, 8 * 2 * R * P], F8, kind="ExternalInput").ap()
    w8 = nc.dram_tensor("w8", [P, NCH * KT * OCH], F8, kind="ExternalInput").ap()
    wr = nc.dram_tensor("wr", [P, NCH * 2 * R * OCH], F8, kind="ExternalInput").ap()
    sc = nc.dram_tensor("sc", [1, 1], F32, kind="ExternalInput").ap()
    bias = nc.dram_tensor("bias", [1, OUT_F], F32, kind="ExternalInput").ap()
    y = nc.dram_tensor("y", [ROWS, OUT_F], F16, kind="ExternalOutput").ap()

    with tile.TileContext(nc) as tc:
        with (
            tc.tile_pool(name="xp", bufs=1) as xp,
            tc.tile_pool(name="wp", bufs=2) as wp,
            tc.tile_pool(name="bp", bufs=2) as bp,
            tc.tile_pool(name="yp", bufs=4) as yp,
            tc.tile_pool(name="psum", bufs=8, space="PSUM") as pp,
        ):
            scb = xp.tile([P, 1], F32)
            xlo = xp.tile([P, KT, HR], F8)          # mi 0-7 halves, tile-major
            xhi = xp.tile([P, 8, KT, P], F8)        # mi 8-15, mi-major
            xrlo = xp.tile([P, 2 * R, HR], F8)
            xrhi = xp.tile([P, 8, 2 * R, P], F8)
            pss = [pp.tile([P, OCH], F32, name="ps") for mi in range(8)]
            gatet = xp.tile([1, 1], F8)

            def lhs(m, i, lo, hi):
                """x slice [P, 2, 128] for m-tile m, k-pair i."""
                if m < 8:
                    return lo[:, 2 * i : 2 * i + 2, ts(m, P)]
                return hi[:, m - 8, 2 * i : 2 * i + 2, :]

            for j in range(NCH):
                jo = j * OCH
                wt_j = wp.tile([P, KT, OCH], F8)
                wr_j = wp.tile([P, 2 * R, OCH], F8)
                wb = j * KT * OCH
                rb = j * 2 * R * OCH
                if j == 0:
                    # head feed in consumption order, bytes balanced across
                    # the two HWDGE queues. Pair 0 ships in quarter-tiles so
                    # the first matmul's deps are only ~128KB per queue; the
                    # later pairs go as single [P, 2048] DMAs (2KB-contiguous
                    # per-partition packets — per-ring rate is packet-bound).
                    nc.scalar.dma_start(
                        out=wt_j[:, 0:2, :], in_=w8[:, wb : wb + 2 * OCH]
                    )
                    nc.sync.dma_start(
                        out=xlo[:, 0, ts(0, 512)], in_=x8a[:, 0:512]
                    )
                    nc.sync.dma_start(
                        out=xlo[:, 1, ts(0, 512)], in_=x8a[:, HR : HR + 512]
                    )
                    nc.scalar.dma_start(
                        out=xlo[:, 0, ts(1, 512)], in_=x8a[:, 512:1024]
                    )
                    nc.scalar.dma_start(
                        out=xlo[:, 1, ts(1, 512)],
                        in_=x8a[:, HR + 512 : HR + 1024],
                    )
                    # pair 1 alone, then quads of 4 k-tiles (4KB-contiguous
                    # per-partition packets lift ring rate to ~300 GB/s so
                    # the k-major phase's 225 GB/s demand never starves);
                    # w pairs ride the opposite ring.
                    nc.sync.dma_start(
                        out=xlo[:, 2:4, :], in_=x8a[:, 2 * HR : 4 * HR]
                    )
                    nc.scalar.dma_start(
                        out=wt_j[:, 2:4, :],
                        in_=w8[:, wb + 2 * OCH : wb + 4 * OCH],
                    )
                    for q in range(1, KP // 2):
                        xq = nc.sync if q % 2 else nc.scalar
                        xq.dma_start(
                            out=xlo[:, 4 * q : 4 * q + 4, :],
                            in_=x8a[:, 4 * q * HR : (4 * q + 4) * HR],
                        )
                        if q == 1:
                            # w0 k-tiles 4..15 in two 6KB-packet halves on
                            # the ring opposite the x quads' cadence
                            nc.scalar.dma_start(
                                out=wt_j[:, 4:16, :],
                                in_=w8[:, wb + 4 * OCH : wb + 16 * OCH],
                            )
                        elif q == 4:
                            nc.sync.dma_start(
                                out=wt_j[:, 16:32, :],
                                in_=w8[:, wb + 16 * OCH : wb + 32 * OCH],
                            )
                    for q in range(R // 2):
                        xq = nc.scalar if q % 2 else nc.sync
                        xq.dma_start(
                            out=xrlo[:, 4 * q : 4 * q + 4, :],
                            in_=xra[:, 4 * q * HR : (4 * q + 4) * HR],
                        )
                        if q == 0:
                            nc.sync.dma_start(
                                out=wr_j[:, 0 : R, :],
                                in_=wr[:, rb : rb + R * OCH],
                            )
                        elif q == 2:
                            nc.scalar.dma_start(
                                out=wr_j[:, R : 2 * R, :],
                                in_=wr[:, rb + R * OCH : rb + 2 * R * OCH],
                            )
                    # needed by the first drain (~45us in)
                    nc.scalar.dma_start(
                        out=scb, in_=sc[0:1, 0:1].broadcast_to([P, 1])
                    )
                    # mi 8-15 slices ride the independent SWDGE (gpsimd)
                    # ring, gated behind the last head-feed tile (a 1-byte
                    # SBUF->SBUF copy of wr pair 7) so their transfers start
                    # only once the k-major head stream is done (~40us) and
                    # don't contend with it for HBM; they're consumed from
                    # ~55us (mi=8) through ~92us (mi=15).
                    nc.gpsimd.dma_start(out=gatet, in_=xrlo[0:1, 0, 0:1])
                    for mi in range(8):
                        nc.gpsimd.dma_start(
                            out=xhi[:, mi],
                            in_=x8b[:, mi * KT * P : (mi + 1) * KT * P],
                        )
                        nc.gpsimd.dma_start(
                            out=xrhi[:, mi],
                            in_=xrb[:, mi * 2 * R * P : (mi + 1) * 2 * R * P],
                        )
                else:
                    # steady state: one big linear DMA per stream (16KB and
                    # 8KB per-partition packets), all on the scalar queue
                    nc.scalar.dma_start(out=wt_j, in_=w8[:, wb : wb + KT * OCH])
                    nc.scalar.dma_start(
                        out=wr_j, in_=wr[:, rb : rb + 2 * R * OCH]
                    )
                bt = bp.tile([P, OCH], F32)
                nc.scalar.dma_start(
                    out=bt, in_=bias[0:1, jo : jo + OCH].broadcast_to([P, OCH])
                )

                def _drain(ps, m):
                    ysb = yp.tile([P, OCH], F16, name="ysb")
                    # fused drain: ysb = psum * scale + bias
                    nc.vector.scalar_tensor_tensor(
                        out=ysb,
                        in0=ps,
                        scalar=scb,
                        in1=bt,
                        op0=mybir.AluOpType.mult,
                        op1=mybir.AluOpType.add,
                    )
                    nc.sync.dma_start(out=y[ts(m, P), jo : jo + OCH], in_=ysb)

                if j == 0:
                    # chunk 0 overlaps the initial feed: m-tiles 0..7 go
                    # k-major across 8 PSUM banks (pss, allocated above) so
                    # the PE consumes each k-pair as it lands; m-tiles 8..15
                    # then go m-major (all data resident, and groups hand
                    # banks over one at a time instead of stalling on 8
                    # serialized drains).
                    for i in range(KP):
                        for mi in range(8):
                            nc.tensor.matmul(
                                pss[mi],
                                lhs(mi, i, xlo, xhi),
                                wt_j[:, 2 * i : 2 * i + 2, :],
                                start=(i == 0),
                                stop=False,
                                perf_mode=DR,
                            )
                    for i in range(R):
                        for mi in range(8):
                            nc.tensor.matmul(
                                pss[mi],
                                lhs(mi, i, xrlo, xrhi),
                                wr_j[:, 2 * i : 2 * i + 2, :],
                                start=False,
                                stop=(i == R - 1),
                                perf_mode=DR,
                            )
                    for mi in range(8):
                        _drain(pss[mi], mi)
                mrange = range(8, MT) if j == 0 else range(MT)
                for m in mrange:
                    ps = pp.tile([P, OCH], F32, name="ps")
                    for i in range(KP):
                        nc.tensor.matmul(
                            ps,
                            lhs(m, i, xlo, xhi),
                            wt_j[:, 2 * i : 2 * i + 2, :],
                            start=(i == 0),
                            stop=False,
                            perf_mode=DR,
                        )
                    for i in range(R):
                        nc.tensor.matmul(
                            ps,
                            lhs(m, i, xrlo, xrhi),
                            wr_j[:, 2 * i : 2 * i + 2, :],
                            start=False,
                            stop=(i == R - 1),
                            perf_mode=DR,
                        )
                    _drain(ps, m)

    nc.compile()
    return nc


def _get_nc():
    if "nc" not in _NC_CACHE:
        _NC_CACHE["nc"] = _build()
    return _NC_CACHE["nc"]


def _ref_scale(weight):
    """max(mean(|W|), 1e-8) bit-exactly as the jax reference computes it."""
    import jax
    import jax.numpy as jnp

    with jax.default_device(jax.devices("cpu")[0]):
        s = jnp.maximum(jnp.mean(jnp.abs(weight)), 1e-8)
        return np.float32(np.asarray(s))


def _split_lo_hi(a_t, nt):
    """[nt*P, ROWS] (k on rows) -> (lo [P, nt*HR] tile-major halves,
    hi [P, 8*nt*P] mi-major 128-col slices)."""
    tiles = a_t.reshape(nt, P, ROWS)
    lo = np.ascontiguousarray(
        tiles[:, :, :HR].transpose(1, 0, 2).reshape(P, nt * HR)
    )
    hi = np.ascontiguousarray(
        tiles[:, :, HR:]
        .reshape(nt, P, 8, P)
        .transpose(1, 2, 0, 3)
        .reshape(P, 8 * nt * P)
    )
    return lo, hi


def kernel(x, weight, bias):
    global LAST_RESULTS
    x = np.asarray(x)
    weight = np.asarray(weight, dtype=np.float32)
    bias = np.asarray(bias, dtype=np.float32)
    b, s, _ = x.shape
    rows = b * s
    assert rows == N_CORES * ROWS

    scale = _ref_scale(weight)
    # with the exact scale, numpy round/clip match the reference ternary
    tern = np.clip(np.round(weight / scale), -1.0, 1.0).astype(np.float32)
    tt = tern.T.astype(E4)                                     # [in, out] +-1
    # w8: [P, NCH*KT*OCH] — chunk-major then k-tile, linear per partition
    w8 = np.ascontiguousarray(
        tt.reshape(KT, P, NCH, OCH).transpose(1, 2, 0, 3).reshape(P, -1)
    )
    wrm = np.ascontiguousarray(
        (tern.T[: 2 * R * P] * np.float32(0.0625))
        .astype(E4)
        .reshape(2 * R, P, NCH, OCH)
        .transpose(1, 2, 0, 3)
        .reshape(P, -1)
    )
    sc = np.full((1, 1), scale, dtype=np.float32)
    b2 = np.ascontiguousarray(bias.reshape(1, OUT_F))

    xf = x.reshape(rows, IN_F).astype(np.float32)
    in_maps = []
    for c in range(N_CORES):
        xs = xf[c * ROWS : (c + 1) * ROWS]
        x8c = xs.astype(E4)
        xr16 = ((xs - x8c.astype(np.float32)) * np.float32(16.0)).astype(E4)
        lo, hi = _split_lo_hi(np.ascontiguousarray(x8c.T), KT)
        rlo, rhi = _split_lo_hi(
            np.ascontiguousarray(xr16.T[: 2 * R * P]), 2 * R
        )
        in_maps.append(
            {
                "x8a": lo,
                "x8b": hi,
                "xra": rlo,
                "xrb": rhi,
                "w8": w8,
                "wr": wrm,
                "sc": sc,
                "bias": b2,
            }
        )

    nc = _get_nc()
    try:
        res = run_bass_kernel_spmd(nc, in_maps, core_ids=list(range(N_CORES)))
    except Exception:
        # transient device wedge (NRT_EXEC_UNIT_UNRECOVERABLE) — one retry
        import time

        time.sleep(5.0)
        res = run_bass_kernel_spmd(nc, in_maps, core_ids=list(range(N_CORES)))
    LAST_RESULTS = res
    y = np.concatenate([res.results[c]["y"] for c in range(N_CORES)], axis=0)
    return np.ascontiguousarray(y.reshape(b, s, OUT_F).astype(np.float32))


# revision 17
# speedup vs baseline: 1.0060x; 1.0060x over previous
"""BitLinear (BitNet b1.58) forward kernel for Trainium2, 8 NeuronCores.

Computes  y = einsum('bsi,oi->bso', x, w_ste) + bias  where
  scale  = max(mean(|W|), 1e-8)
  w_q    = clip(round(W/scale), -1, 1)   (ternary)
  w_ste  = w_q * scale

Sharding: data-parallel over rows; each core owns one batch element
(2048 rows) and the full weight.

Quantization happens on the HOST, bit-exactly replicating the reference
(scale via jax-on-CPU mean — numpy's pairwise mean is 2 ulps off, which
flips ternary weights at the round(w/scale) boundary; with the exact
scale, numpy's round/clip reproduce the reference ternary identically).

Device: pure fp8 DoubleRow matmuls (2 contraction rows/cycle — the only
2x-rate PE mode; requires both operands fp8e4/e5; measured 216ns per
512-out-row MM = the 157 TF/s fp8 peak). All error then comes from
e4m3-quantizing x, reduced by a residual pass over the first R of 16
k-pairs: xr16 = e4m3(16*(x - x8)) against wr = ternary*(1/16),
accumulated into the same PSUM group. Host-simulated (exact predictor
of hw, verified): rel = 1.893e-2 at R=8 vs the 2e-2 gate.

Schedule: the PE floor is (16+R)/16 * 437us; everything else hides
behind it. Chunk 0 runs k-major across 8 PSUM banks (m-tiles 0-7) so
the PE starts as soon as k-pair 0 lands. To keep the k-major phase
under the per-core DMA roofline (~358 GB/s), x ships in two DRAM
tensors: x8a holds the mi 0-7 halves of each k-tile (tile-major, fed
in consumption order during the k-major phase at ~220 GB/s demand),
and x8b holds the mi 8-15 slices in MI-MAJOR layout so each m-tile of
the later m-major phase arrives as one 512KB DMA with 4KB-contiguous
per-partition packets. xr splits the same way (xra/xrb).

Head DMAs are byte-balanced across the two hardware DGE queues
(SP=nc.sync and Activation=nc.scalar) in consumption order, with the
x8b/xrb (mi 8-15) streams on the independent SWDGE ring (nc.gpsimd) so
the three descriptor streams together approach the per-core HBM rate;
steady state specializes (scalar: one big weight DMA per chunk; sync:
y writes). Drain: y16 = psum * scale + bias fused on DVE, written fp16
(upcast to f32 on host; fp16 rounding adds ~2e-4 rel).
"""

import numpy as np
import ml_dtypes

import concourse.tile as tile
import concourse.mybir as mybir
from concourse import bacc
from concourse.bass import ts
from concourse.bass_utils import run_bass_kernel_spmd

N_CORES = 8
IN_F = 4096
OUT_F = 4096
ROWS = 2048               # rows per core
P = 128                   # SBUF partitions
KT = IN_F // P            # 32 k-tiles
KP = KT // 2              # 16 k-pairs (DoubleRow covers 2 k-tiles)
MT = ROWS // P            # 16 row-tiles per core
OCH = 512                 # out-feature chunk = PSUM bank width
NCH = OUT_F // OCH        # 8 chunks
R = 8                     # residual k-pairs covered (of 16)
HR = ROWS // 2            # 1024: columns of the mi 0-7 half of a k-tile

F32 = mybir.dt.float32
F16 = mybir.dt.float16
F8 = mybir.dt.float8e4
E4 = np.dtype(ml_dtypes.float8_e4m3)
DR = mybir.MatmulPerfMode.DoubleRow

LAST_RESULTS = None
_NC_CACHE = {}


def _build():
    nc = bacc.Bacc(
        "TRN2", target_bir_lowering=False, debug=False, num_devices=N_CORES
    )
    # partition-major layouts (second dim is per-partition linear bytes)
    x8a = nc.dram_tensor("x8a", [P, KT * HR], F8, kind="ExternalInput").ap()
    x8b = nc.dram_tensor("x8b", [P, 8 * KT * P], F8, kind="ExternalInput").ap()
    xra = nc.dram_tensor("xra", [P, 2 * R * HR], F8, kind="ExternalInput").ap()
    xrb = nc.dram_tensor("xrb", [P, 8 * 2 * R * P], F8, kind="ExternalInput").ap()
    w8 = nc.dram_tensor("w8", [P, NCH * KT * OCH], F8, kind="ExternalInput").ap()
    wr = nc.dram_tensor("wr", [P, NCH * 2 * R * OCH], F8, kind="ExternalInput").ap()
    sc = nc.dram_tensor("sc", [1, 1], F32, kind="ExternalInput").ap()
    bias = nc.dram_tensor("bias", [1, OUT_F], F32, kind="ExternalInput").ap()
    y = nc.dram_tensor("y", [ROWS, OUT_F], F16, kind="ExternalOutput").ap()

    with tile.TileContext(nc) as tc:
        with (
            tc.tile_pool(name="xp", bufs=1) as xp,
            tc.tile_pool(name="wp", bufs=2) as wp,
            tc.tile_pool(name="bp", bufs=2) as bp,
            tc.tile_pool(name="yp", bufs=4) as yp,
            tc.tile_pool(name="psum", bufs=8, space="PSUM") as pp,
        ):
            scb = xp.tile([P, 1], F32)
            xlo = xp.tile([P, KT, HR], F8)          # mi 0-7 halves, tile-major
            xhi = xp.tile([P, 8, KT, P], F8)        # mi 8-15, mi-major
            xrlo = xp.tile([P, 2 * R, HR], F8)
            xrhi = xp.tile([P, 8, 2 * R, P], F8)
            pss = [pp.tile([P, OCH], F32, name="ps") for mi in range(8)]
            gatet = xp.tile([1, 1], F8)

            def lhs(m, i, lo, hi):
                """x slice [P, 2, 128] for m-tile m, k-pair i."""
                if m < 8:
                    return lo[:, 2 * i : 2 * i + 2, ts(m, P)]
                return hi[:, m - 8, 2 * i : 2 * i + 2, :]

            for j in range(NCH):
                jo = j * OCH
                wt_j = wp.tile([P, KT, OCH], F8)
                wr_j = wp.tile([P, 2 * R, OCH], F8)
                wb = j * KT * OCH
                rb = j * 2 * R * OCH
                if j == 0:
                    # -- head feed, consumption order, three descriptor
                    # streams. The SWDGE (gpsimd) queue boots ~2us before
                    # the HWDGE rings' first transfer, so it carries the
                    # first two k-pairs (x tiles 0-3 + w pairs 0-1) for the
                    # earliest possible PE start.
                    nc.gpsimd.dma_start(
                        out=wt_j[:, 0:2, :], in_=w8[:, wb : wb + 2 * OCH]
                    )
                    nc.gpsimd.dma_start(out=xlo[:, 0, :], in_=x8a[:, 0:HR])
                    nc.gpsimd.dma_start(
                        out=xlo[:, 1, :], in_=x8a[:, HR : 2 * HR]
                    )
                    nc.gpsimd.dma_start(
                        out=xlo[:, 2:4, :], in_=x8a[:, 2 * HR : 4 * HR]
                    )
                    nc.gpsimd.dma_start(
                        out=wt_j[:, 2:4, :],
                        in_=w8[:, wb + 2 * OCH : wb + 4 * OCH],
                    )
                    # x quads of 4 k-tiles (4KB-contiguous per-partition
                    # packets) on sync; w0 and the residual feed on scalar.
                    for q in range(1, KP // 2):
                        nc.sync.dma_start(
                            out=xlo[:, 4 * q : 4 * q + 4, :],
                            in_=x8a[:, 4 * q * HR : (4 * q + 4) * HR],
                        )
                    nc.scalar.dma_start(
                        out=wt_j[:, 4:16, :],
                        in_=w8[:, wb + 4 * OCH : wb + 16 * OCH],
                    )
                    nc.scalar.dma_start(
                        out=wt_j[:, 16:32, :],
                        in_=w8[:, wb + 16 * OCH : wb + 32 * OCH],
                    )
                    for q in range(R // 2):
                        (nc.scalar if q % 2 else nc.sync).dma_start(
                            out=xrlo[:, 4 * q : 4 * q + 4, :],
                            in_=xra[:, 4 * q * HR : (4 * q + 4) * HR],
                        )
                    nc.scalar.dma_start(
                        out=wr_j[:, 0:R, :], in_=wr[:, rb : rb + R * OCH]
                    )
                    nc.scalar.dma_start(
                        out=wr_j[:, R : 2 * R, :],
                        in_=wr[:, rb + R * OCH : rb + 2 * R * OCH],
                    )
                    # needed by the first drain (~45us in)
                    nc.scalar.dma_start(
                        out=scb, in_=sc[0:1, 0:1].broadcast_to([P, 1])
                    )
                    # mi 8-15 slices continue on the SWDGE queue, gated
                    # behind the second wr half (lands ~30us) so their
                    # transfers don't contend with the k-major head; they
                    # are consumed from ~54us (mi=8) through ~91us (mi=15).
                    nc.gpsimd.dma_start(out=gatet, in_=wr_j[0:1, R, 0:1])
                    for mi in range(8):
                        nc.gpsimd.dma_start(
                            out=xhi[:, mi],
                            in_=x8b[:, mi * KT * P : (mi + 1) * KT * P],
                        )
                        nc.gpsimd.dma_start(
                            out=xrhi[:, mi],
                            in_=xrb[:, mi * 2 * R * P : (mi + 1) * 2 * R * P],
                        )
                else:
                    # steady state: one big linear DMA per stream (16KB and
                    # 8KB per-partition packets), all on the scalar queue
                    nc.scalar.dma_start(out=wt_j, in_=w8[:, wb : wb + KT * OCH])
                    nc.scalar.dma_start(
                        out=wr_j, in_=wr[:, rb : rb + 2 * R * OCH]
                    )
                bt = bp.tile([P, OCH], F32)
                nc.scalar.dma_start(
                    out=bt, in_=bias[0:1, jo : jo + OCH].broadcast_to([P, OCH])
                )

                def _drain(ps, m):
                    ysb = yp.tile([P, OCH], F16, name="ysb")
                    # fused drain: ysb = psum * scale + bias
                    nc.vector.scalar_tensor_tensor(
                        out=ysb,
                        in0=ps,
                        scalar=scb,
                        in1=bt,
                        op0=mybir.AluOpType.mult,
                        op1=mybir.AluOpType.add,
                    )
                    nc.sync.dma_start(out=y[ts(m, P), jo : jo + OCH], in_=ysb)

                if j == 0:
                    # chunk 0 overlaps the initial feed: m-tiles 0..7 go
                    # k-major across 8 PSUM banks (pss, allocated above) so
                    # the PE consumes each k-pair as it lands; m-tiles 8..15
                    # then go m-major (all data resident, and groups hand
                    # banks over one at a time instead of stalling on 8
                    # serialized drains).
                    for i in range(KP):
                        for mi in range(8):
                            nc.tensor.matmul(
                                pss[mi],
                                lhs(mi, i, xlo, xhi),
                                wt_j[:, 2 * i : 2 * i + 2, :],
                                start=(i == 0),
                                stop=False,
                                perf_mode=DR,
                            )
                    for i in range(R):
                        for mi in range(8):
                            nc.tensor.matmul(
                                pss[mi],
                                lhs(mi, i, xrlo, xrhi),
                                wr_j[:, 2 * i : 2 * i + 2, :],
                                start=False,
                                stop=(i == R - 1),
                                perf_mode=DR,
                            )
                    for mi in range(8):
                        _drain(pss[mi], mi)
                mrange = range(8, MT) if j == 0 else range(MT)
                for m in mrange:
                    ps = pp.tile([P, OCH], F32, name="ps")
                    for i in range(KP):
                        nc.tensor.matmul(
                            ps,
                            lhs(m, i, xlo, xhi),
                            wt_j[:, 2 * i : 2 * i + 2, :],
                            start=(i == 0),
                            stop=False,
                            perf_mode=DR,
                        )
                    for i in range(R):
                        nc.tensor.matmul(
                            ps,
                            lhs(m, i, xrlo, xrhi),
                            wr_j[:, 2 * i : 2 * i + 2, :],
                            start=False,
                            stop=(i == R - 1),
                            perf_mode=DR,
                        )
                    _drain(ps, m)

    nc.compile()
    return nc


def _get_nc():
    if "nc" not in _NC_CACHE:
        _NC_CACHE["nc"] = _build()
    return _NC_CACHE["nc"]


def _ref_scale(weight):
    """max(mean(|W|), 1e-8) bit-exactly as the jax reference computes it."""
    import jax
    import jax.numpy as jnp

    with jax.default_device(jax.devices("cpu")[0]):
        s = jnp.maximum(jnp.mean(jnp.abs(weight)), 1e-8)
        return np.float32(np.asarray(s))


def _split_lo_hi(a_t, nt):
    """[nt*P, ROWS] (k on rows) -> (lo [P, nt*HR] tile-major halves,
    hi [P, 8*nt*P] mi-major 128-col slices)."""
    tiles = a_t.reshape(nt, P, ROWS)
    lo = np.ascontiguousarray(
        tiles[:, :, :HR].transpose(1, 0, 2).reshape(P, nt * HR)
    )
    hi = np.ascontiguousarray(
        tiles[:, :, HR:]
        .reshape(nt, P, 8, P)
        .transpose(1, 2, 0, 3)
        .reshape(P, 8 * nt * P)
    )
    return lo, hi


def kernel(x, weight, bias):
    global LAST_RESULTS
    x = np.asarray(x)
    weight = np.asarray(weight, dtype=np.float32)
    bias = np.asarray(bias, dtype=np.float32)
    b, s, _ = x.shape
    rows = b * s
    assert rows == N_CORES * ROWS

    scale = _ref_scale(weight)
    # with the exact scale, numpy round/clip match the reference ternary
    tern = np.clip(np.round(weight / scale), -1.0, 1.0).astype(np.float32)
    tt = tern.T.astype(E4)                                     # [in, out] +-1
    # w8: [P, NCH*KT*OCH] — chunk-major then k-tile, linear per partition
    w8 = np.ascontiguousarray(
        tt.reshape(KT, P, NCH, OCH).transpose(1, 2, 0, 3).reshape(P, -1)
    )
    wrm = np.ascontiguousarray(
        (tern.T[: 2 * R * P] * np.float32(0.0625))
        .astype(E4)
        .reshape(2 * R, P, NCH, OCH)
        .transpose(1, 2, 0, 3)
        .reshape(P, -1)
    )
    sc = np.full((1, 1), scale, dtype=np.float32)
    b2 = np.ascontiguousarray(bias.reshape(1, OUT_F))

    xf = x.reshape(rows, IN_F).astype(np.float32)
    in_maps = []
    for c in range(N_CORES):
        xs = xf[c * ROWS : (c + 1) * ROWS]
        x8c = xs.astype(E4)
        xr16 = ((xs - x8c.astype(np.float32)) * np.float32(16.0)).astype(E4)
        lo, hi = _split_lo_hi(np.ascontiguousarray(x8c.T), KT)
        rlo, rhi = _split_lo_hi(
            np.ascontiguousarray(xr16.T[: 2 * R * P]), 2 * R
        )
        in_maps.append(
            {
                "x8a": lo,
                "x8b": hi,
                "xra": rlo,
                "xrb": rhi,
                "w8": w8,
                "wr": wrm,
                "sc": sc,
                "bias": b2,
            }
        )

    nc = _get_nc()
    try:
        res = run_bass_kernel_spmd(nc, in_maps, core_ids=list(range(N_CORES)))
    except Exception:
        # transient device wedge (NRT_EXEC_UNIT_UNRECOVERABLE) — one retry
        import time

        time.sleep(5.0)
        res = run_bass_kernel_spmd(nc, in_maps, core_ids=list(range(N_CORES)))
    LAST_RESULTS = res
    y = np.concatenate([res.results[c]["y"] for c in range(N_CORES)], axis=0)
    return np.ascontiguousarray(y.reshape(b, s, OUT_F).astype(np.float32))
